# revision 6
# baseline (speedup 1.0000x reference)
"""Trainium2 Bass kernel for nn_Critic GNN message-passing critic.

Problem (hardcoded shapes): B=1024 graphs x 64 nodes x 4 feats, 1024 edges/graph
(same topology per graph), EdgeConv MLP 10->32->32, scatter-add by src, then a
per-edge critic head 73->32->1 summed over 1027 rows per graph.

Strategy: data-parallel over graphs, 128 graphs per NeuronCore x 8 cores.
All gathers/scatters become matmuls against one-hot matrices built on the host
from the runtime index tensors. W2 is folded through the segment-sum
(segment_sum(relu(.) @ W2) == segment_sum(relu(.)) @ W2), so the second MLP
layer collapses into the phase-2 node tables.
"""

import numpy as np
from contextlib import ExitStack

from concourse import bass, bacc, tile, bass_utils
from concourse import mybir

f32 = mybir.dt.float32
RELU = mybir.ActivationFunctionType.Relu
MAX = mybir.AluOpType.max
MULT = mybir.AluOpType.mult
ADD = mybir.AluOpType.add

# ---- problem constants ----
B, NN, NODE, EDGEF, HID, NFACT, NE = 1024, 64, 4, 2, 32, 3, 1024
NCORES = 8
GPC = B // NCORES          # 128 graphs per core
NTG = GPC // 16            # 8 groups of 16 graphs
NSG = GPC // 4             # 32 subgroups of 4 graphs
EC = NE // 128             # 8 edge chunks of 128
E2 = NE + NFACT            # 1027
E2P = 1152                 # padded to 9*128
SPLIT2 = 576               # phase-2 relu/accum column split (ACT|DVE)

_CACHE = {}


def _build_nc(debug=False):
    nc = bacc.Bacc("TRN2", target_bir_lowering=False, debug=False,
                   num_devices=NCORES)

    def din(name, shape):
        return nc.dram_tensor(name, shape, f32, kind="ExternalInput").ap()

    # per-core data
    xT = din("xT", [64, 64 * NTG])          # [(16g,4f), n] per 16-graph group
    xT2 = din("xT2", [16, 64 * NSG])        # [(4g,4f), n] per subgroup (base-0)
    eaT = din("eaT", [33, 128 * NTG * EC])  # [(16g,2c)+ones, e] tiles
    Gt = din("Gt", [128, NE])               # one-hot gather (src|dst) columns=e
    St = din("St", [128, 64 * EC])          # one-hot scatter chunks
    G2t = din("G2t", [128, E2P])            # phase-2 gather, zero-padded cols
    actB = din("actB", [96, 2 * E2P])       # banded action rows (base 0/32/64)
    selP = din("selP", [96, 128 * 8])       # banded wl_c selectors
    blcol = din("blcol", [128, 1])          # bl[j] per (g,j) partition
    # constants (same on all cores)
    W1a_blk = din("W1a_blk", [64, 512])
    W1b_blk = din("W1b_blk", [64, 512])
    W1cb = din("W1cb", [33, 512])
    Wla4_blk = din("Wla4_blk", [16, 128])
    Wlap_blk = din("Wlap_blk", [128, 128])
    Wlb4_blk = din("Wlb4_blk", [16, 128])
    Wlbp_blk = din("Wlbp_blk", [128, 128])
    V2corr = din("V2corr", [128, 128])      # c_n * b2-fold correction (zeros if b2==0)
    ident = din("ident", [64, 64])
    WvP = din("WvP", [128, 4])
    vout = nc.dram_tensor("v", [4, 2 * NSG], f32, kind="ExternalOutput").ap()
    dbg = {}
    if debug:
        for name, shape in [("dbg_V1", [128, 512 * NTG]), ("dbg_U", [64, 512 * NTG]),
                            ("dbg_UT", [128, 64 * NSG]), ("dbg_V2", [128, 128 * NSG]),
                            ("dbg_S1", [128, 2 * NSG])]:
            dbg[name] = nc.dram_tensor(name, shape, f32, kind="ExternalOutput").ap()

    with tile.TileContext(nc) as tc:
        with ExitStack() as ctx:
            cpool = ctx.enter_context(tc.tile_pool(name="consts", bufs=1))

            def load(ap, shape, tag):
                t = cpool.tile(shape, f32, tag=tag)
                nc.sync.dma_start(t[:], ap[:])
                return t

            t_xT = load(xT, [64, 64 * NTG], "xT")
            t_xT2 = load(xT2, [16, 64 * NSG], "xT2")
            t_eaT = load(eaT, [33, 128 * NTG * EC], "eaT")
            t_Gt = load(Gt, [128, NE], "Gt")
            t_St = load(St, [128, 64 * EC], "St")
            t_G2t = load(G2t, [128, E2P], "G2t")
            t_actB = load(actB, [96, 2 * E2P], "actB")
            t_selP = load(selP, [96, 128 * 8], "selP")
            t_blc = load(blcol, [128, 1], "blcol")
            t_W1a = load(W1a_blk, [64, 512], "W1a")
            t_W1b = load(W1b_blk, [64, 512], "W1b")
            t_W1cb = load(W1cb, [33, 512], "W1cb")
            t_Wla4 = load(Wla4_blk, [16, 128], "Wla4")
            t_Wlap = load(Wlap_blk, [128, 128], "Wlap")
            t_Wlb4 = load(Wlb4_blk, [16, 128], "Wlb4")
            t_Wlbp = load(Wlbp_blk, [128, 128], "Wlbp")
            t_V2c = load(V2corr, [128, 128], "V2c")
            t_id = load(ident, [64, 64], "ident")
            t_WvP = load(WvP, [128, 4], "WvP")

            # persistent SBUF intermediates
            t_V1 = cpool.tile([128, 512 * NTG], f32, tag="V1")     # [slots,(16g,32j)]
            t_U = cpool.tile([64, 512 * NTG], f32, tag="U")        # [n,(16g,32j)]
            t_UT = cpool.tile([128, 64 * NSG], f32, tag="UT")      # [(4g,32jj), n]
            t_V2 = cpool.tile([128, 128 * NSG], f32, tag="V2")     # [slots,(4g,32j)]
            t_S1 = cpool.tile([128, 2 * NSG], f32, tag="S1")       # relu-sum accums

            # ---------------- phase A: V1 = [x@W1a ; x@W1b] ----------------
            with tc.tile_pool(name="psA", bufs=2, space=bass.MemorySpace.PSUM) as psA:
                for tg in range(NTG):
                    pv = psA.tile([128, 512], f32, tag="pv")
                    lx = t_xT[:, tg * 64:(tg + 1) * 64]
                    nc.tensor.matmul(pv[0:64, :], lx, t_W1a[:], start=True, stop=True)
                    nc.tensor.matmul(pv[64:128, :], lx, t_W1b[:], start=True, stop=True)
                    dst = t_V1[:, tg * 512:(tg + 1) * 512]
                    nc.scalar.copy(dst[:, 0:256], pv[:, 0:256])
                    nc.vector.tensor_copy(dst[:, 256:512], pv[:, 256:512])

            # ---------------- phase B: pre1 -> relu -> U ----------------
            with tc.tile_pool(name="psB", bufs=3, space=bass.MemorySpace.PSUM) as psB, \
                 tc.tile_pool(name="psU", bufs=2, space=bass.MemorySpace.PSUM) as psU, \
                 tc.tile_pool(name="relu1", bufs=4) as rpool:
                for tg in range(NTG):
                    pu = psU.tile([64, 512], f32, tag="pu")
                    for c in range(EC):
                        p1 = psB.tile([128, 512], f32, tag="p1")
                        gt = t_Gt[:, c * 128:(c + 1) * 128]
                        v1 = t_V1[:, tg * 512:(tg + 1) * 512]
                        nc.tensor.matmul(p1[:], gt, v1, start=True, stop=False)
                        ea = t_eaT[:, (tg * EC + c) * 128:(tg * EC + c + 1) * 128]
                        nc.tensor.matmul(p1[:], ea, t_W1cb[:], start=False, stop=True)
                        r1 = rpool.tile([128, 512], f32, tag="r1")
                        nc.scalar.activation(r1[:, 0:256], p1[:, 0:256], RELU)
                        nc.vector.tensor_scalar_max(r1[:, 256:512], p1[:, 256:512], 0.0)
                        st = t_St[:, c * 64:(c + 1) * 64]
                        nc.tensor.matmul(pu[:], st, r1[:],
                                         start=(c == 0), stop=(c == EC - 1))
                    dst = t_U[:, tg * 512:(tg + 1) * 512]
                    nc.scalar.copy(dst[:, 0:256], pu[:, 0:256])
                    nc.vector.tensor_copy(dst[:, 256:512], pu[:, 256:512])

            # ---------------- phase C: U^T, V2 tables ----------------
            with tc.tile_pool(name="psT", bufs=2, space=bass.MemorySpace.PSUM) as psT, \
                 tc.tile_pool(name="psV2", bufs=2, space=bass.MemorySpace.PSUM) as psV2:
                for tg in range(NTG):
                    pt = psT.tile([128, 256], f32, tag="pt")
                    for sl in range(4):
                        blk = t_U[:, tg * 512 + sl * 128: tg * 512 + (sl + 1) * 128]
                        nc.tensor.transpose(pt[:, sl * 64:(sl + 1) * 64], blk, t_id[:])
                    dst = t_UT[:, tg * 256:(tg + 1) * 256]
                    nc.scalar.copy(dst[:, 0:128], pt[:, 0:128])
                    nc.vector.tensor_copy(dst[:, 128:256], pt[:, 128:256])
                for sg in range(NSG):
                    tg, sl = sg // 4, sg % 4
                    pv2 = psV2.tile([128, 128], f32, tag="pv2")
                    lx = t_xT2[:, sg * 64:(sg + 1) * 64]
                    ut = t_UT[:, sg * 64:(sg + 1) * 64]
                    nc.tensor.matmul(pv2[0:64, :], lx, t_Wla4[:], start=True, stop=False)
                    nc.tensor.matmul(pv2[0:64, :], ut, t_Wlap[:], start=False, stop=True)
                    nc.tensor.matmul(pv2[64:128, :], lx, t_Wlb4[:], start=True, stop=False)
                    nc.tensor.matmul(pv2[64:128, :], ut, t_Wlbp[:], start=False, stop=True)
                    dst = t_V2[:, sg * 128:(sg + 1) * 128]
                    # add the c_n * b2 fold while evacuating
                    nc.vector.scalar_tensor_tensor(
                        dst[:, 0:64], pv2[:, 0:64], 1.0,
                        t_V2c[:, 0:64], MULT, ADD)
                    nc.vector.scalar_tensor_tensor(
                        dst[:, 64:128], pv2[:, 64:128], 1.0,
                        t_V2c[:, 64:128], MULT, ADD)

            # ---------------- phase D: pre2 -> relu-sum ----------------
            with tc.tile_pool(name="psD", bufs=2, space=bass.MemorySpace.PSUM) as psD, \
                 tc.tile_pool(name="scr2", bufs=2) as spool:
                t_z = spool.tile([128, E2P - SPLIT2], f32, tag="zeros")
                nc.gpsimd.memset(t_z[:], 0.0)
                nsplits = [(0, 512), (512, 1024), (1024, E2P)]
                for sg in range(NSG):
                    slot = 1 if sg >= 24 else 0
                    band = (sg // 8) % 3 if slot == 0 else 0
                    p = sg % 8 if slot == 0 else sg - 24
                    p2 = psD.tile([128, E2P], f32, tag="p2")
                    v2 = t_V2[:, sg * 128:(sg + 1) * 128]
                    sel = t_selP[band * 32:(band + 1) * 32, p * 128:(p + 1) * 128]
                    for (a, b) in nsplits:
                        nc.tensor.matmul(p2[:, a:b], v2, t_G2t[:, a:b],
                                         start=True, stop=False)
                        arows = t_actB[band * 32:(band + 1) * 32,
                                       slot * E2P + a: slot * E2P + b]
                        nc.tensor.matmul(p2[:, a:b], sel, arows,
                                         start=False, stop=True)
                    scr = spool.tile([128, E2P], f32, tag="scr")
                    nc.scalar.activation(scr[:, 0:SPLIT2], p2[:, 0:SPLIT2], RELU,
                                         bias=t_blc[:],
                                         accum_out=t_S1[:, 2 * sg:2 * sg + 1])
                    nc.vector.scalar_tensor_tensor(
                        scr[:, SPLIT2:E2P], p2[:, SPLIT2:E2P], t_blc[:], t_z[:],
                        ADD, MAX, accum_out=t_S1[:, 2 * sg + 1:2 * sg + 2])

            if debug:
                for name, t in [("dbg_V1", t_V1), ("dbg_U", t_U), ("dbg_UT", t_UT),
                                ("dbg_V2", t_V2), ("dbg_S1", t_S1)]:
                    nc.sync.dma_start(dbg[name][:], t[:])

            # ---------------- finale: fold Wv ----------------
            with tc.tile_pool(name="psF", bufs=1, space=bass.MemorySpace.PSUM) as psF, \
                 tc.tile_pool(name="fin", bufs=1) as fpool:
                pf = psF.tile([4, 2 * NSG], f32, tag="pf")
                nc.tensor.matmul(pf[:], t_WvP[:], t_S1[:], start=True, stop=True)
                fo = fpool.tile([4, 2 * NSG], f32, tag="fo")
                nc.vector.tensor_copy(fo[:], pf[:])
                nc.sync.dma_start(vout[:], fo[:])

    nc.compile()
    return nc


def _blkdiag(blocks_w, g_count, rows_per_g, cols_per_g, W):
    """out[(g,rows), (g,cols)] = W  block-diagonal replication."""
    out = np.zeros((g_count * rows_per_g, g_count * cols_per_g), np.float32)
    for g in range(g_count):
        out[g * rows_per_g:(g + 1) * rows_per_g,
            g * cols_per_g:(g + 1) * cols_per_g] = W
    return out


def _host_prep(inputs):
    x = np.ascontiguousarray(np.asarray(inputs["x"], np.float32))
    ea = np.ascontiguousarray(np.asarray(inputs["edge_attr"], np.float32))
    act = np.ascontiguousarray(np.asarray(inputs["action"], np.float32))
    es = np.asarray(inputs["edges_src"]).astype(np.int64)
    ed = np.asarray(inputs["edges_dst"]).astype(np.int64)
    W1 = np.asarray(inputs["W1"], np.float32)
    b1 = np.asarray(inputs["b1"], np.float32)
    W2 = np.asarray(inputs["W2"], np.float32)
    b2 = np.asarray(inputs["b2"], np.float32)
    Wl = np.asarray(inputs["Wl"], np.float32)
    bl = np.asarray(inputs["bl"], np.float32)
    Wv = np.asarray(inputs["Wv"], np.float32)
    bv = np.asarray(inputs["bv"], np.float32)

    W1a, W1b, W1c = W1[0:4], W1[4:8], W1[8:10]
    Wla4 = Wl[0:4]
    Wlap = W2 @ Wl[4:36]       # fold W2 into phase-2 src table
    Wlb4 = Wl[36:40]
    Wlbp = W2 @ Wl[40:72]
    wlc = Wl[72]               # [32]

    consts = {}
    consts["W1a_blk"] = _blkdiag(None, 16, 4, 32, W1a)
    consts["W1b_blk"] = _blkdiag(None, 16, 4, 32, W1b)
    w1cb = np.zeros((33, 512), np.float32)
    for g in range(16):
        w1cb[2 * g:2 * g + 2, 32 * g:32 * g + 32] = W1c
        w1cb[32, 32 * g:32 * g + 32] = b1
    consts["W1cb"] = w1cb
    consts["Wla4_blk"] = _blkdiag(None, 4, 4, 32, Wla4)
    consts["Wlap_blk"] = _blkdiag(None, 4, 32, 32, Wlap)
    consts["Wlb4_blk"] = _blkdiag(None, 4, 4, 32, Wlb4)
    consts["Wlbp_blk"] = _blkdiag(None, 4, 32, 32, Wlbp)
    # banded wl_c selectors: for each band (replicated at bases 0/32/64) and
    # position p in band, select the 4 action rows of that subgroup
    selp = np.zeros((96, 128 * 8), np.float32)
    for band in range(3):
        for p in range(8):
            for g in range(4):
                selp[band * 32 + 4 * p + g, p * 128 + 32 * g:p * 128 + 32 * g + 32] = wlc
    consts["selP"] = selp
    blcol = np.zeros((128, 1), np.float32)
    for g in range(4):
        blcol[32 * g:32 * g + 32, 0] = bl
    consts["blcol"] = blcol
    consts["ident"] = np.eye(64, dtype=np.float32)
    wvp = np.zeros((128, 4), np.float32)
    for g in range(4):
        wvp[32 * g:32 * g + 32, g] = Wv[:, 0]
    consts["WvP"] = wvp

    # one-hot gather/scatter matrices (shared topology across graphs)
    gt = np.zeros((128, NE), np.float32)
    gt[es, np.arange(NE)] = 1.0
    gt[64 + ed, np.arange(NE)] += 1.0
    consts["Gt"] = gt
    st = np.zeros((128, 64 * EC), np.float32)
    for c in range(EC):
        st[np.arange(128), c * 64 + es[c * 128:(c + 1) * 128]] = 1.0
    consts["St"] = st
    g2t = np.zeros((128, E2P), np.float32)
    g2t[:, :NE] = gt
    for i in range(NFACT):
        g2t[61 + i, NE + i] = 1.0
        g2t[64 + 61 + i, NE + i] += 1.0
    consts["G2t"] = g2t

    # c_n * b2 correction folded into V2 (x_pp = U@W2 + c_n*b2)
    cn = np.bincount(es, minlength=64).astype(np.float32)  # [64]
    v2c = np.zeros((128, 128), np.float32)
    corr_a = np.outer(cn, b2 @ Wl[4:36])   # [64, 32]
    corr_b = np.outer(cn, b2 @ Wl[40:72])
    for g in range(4):
        v2c[0:64, 32 * g:32 * g + 32] = corr_a
        v2c[64:128, 32 * g:32 * g + 32] = corr_b
    consts["V2corr"] = v2c

    x3 = x.reshape(B, NN, NODE)
    ea4 = ea.reshape(B, NE, EDGEF)
    in_maps = []
    for t in range(NCORES):
        m = dict(consts)
        xs = x3[t * GPC:(t + 1) * GPC]          # [128, 64, 4]
        xT = np.zeros((64, 64 * NTG), np.float32)
        for tg in range(NTG):
            blk = xs[tg * 16:(tg + 1) * 16]     # [16, 64, 4]
            xT[:, tg * 64:(tg + 1) * 64] = blk.transpose(0, 2, 1).reshape(64, 64)
        m["xT"] = xT
        xT2 = np.zeros((16, 64 * NSG), np.float32)
        for sg in range(NSG):
            blk = xs[4 * sg:4 * sg + 4]     # [4, 64, 4]
            xT2[:, sg * 64:(sg + 1) * 64] = blk.transpose(0, 2, 1).reshape(16, 64)
        m["xT2"] = xT2
        eas = ea4[t * GPC:(t + 1) * GPC]        # [128, 1024, 2]
        eaT = np.ones((33, 128 * NTG * EC), np.float32)
        for tg in range(NTG):
            for c in range(EC):
                blk = eas[tg * 16:(tg + 1) * 16, c * 128:(c + 1) * 128]  # [16,128,2]
                col = (tg * EC + c) * 128
                eaT[0:32, col:col + 128] = blk.transpose(0, 2, 1).reshape(32, 128)
        m["eaT"] = eaT
        acs = act[t * GPC:(t + 1) * GPC]        # [128, 1027]
        blob = np.zeros((96, 2 * E2P), np.float32)
        for sg in range(NSG):
            slot = 1 if sg >= 24 else 0
            band = (sg // 8) % 3 if slot == 0 else 0
            p = sg % 8 if slot == 0 else sg - 24
            blob[band * 32 + 4 * p:band * 32 + 4 * p + 4,
                 slot * E2P:slot * E2P + E2] = acs[4 * sg:4 * sg + 4]
        m["actB"] = blob
        in_maps.append(m)
    # 1027*bv plus correction for the 125 padded columns that get relu(bl)
    pad_bias = (E2P - E2) * float(np.maximum(bl, 0.0) @ Wv[:, 0])
    extra = float(E2) * float(bv.reshape(-1)[0]) - pad_bias
    return in_maps, extra


def kernel(**inputs) -> np.ndarray:
    if "nc" not in _CACHE:
        _CACHE["nc"] = _build_nc()
    nc = _CACHE["nc"]
    in_maps, extra = _host_prep(inputs)
    res = bass_utils.run_bass_kernel_spmd(nc, in_maps, list(range(NCORES)))
    out = np.empty((B,), np.float32)
    for t in range(NCORES):
        v = res.results[t]["v"]                 # [4, 2*NSG]
        per = v[:, 0::2] + v[:, 1::2]           # [4, NSG]
        out[t * GPC:(t + 1) * GPC] = per.T.reshape(-1) + extra
    return out
